# revision 10
# baseline (speedup 1.0000x reference)
"""Trainium2 Bass kernel for nn_CasamentoMult, v3 = v1 skeleton + 3 fixes.

Math and tiling identical to kernel.py (v1).  Changes, each measured on HW:
- All y-chunk DMAs move from the scalar HWDGE ring to the gpsimd SWDGE
  queue: the 5 DMA_DIRECT2D issue instructions (~650ns each) plus the
  final acc DMA no longer occupy the ACT engine, which was saturated
  start-to-end in v1.
- Activations merged: chunks 0+1 share one u/vw/q act triple, so the accum
  activation count drops 19 -> 14.  Each accum act costs ~573ns fixed
  overhead (352-cycle init + ~280ns ACTIVATION_READ_ACCUMULATOR).
- p runs as 2 PSUM blocks instead of 3 (PSUM-input acts are slightly
  cheaper, and one fewer accum read).
Output acc is [128, 14]: 4 (u,vw,q) triples + 2 p columns.
"""

import math
import numpy as np

ROWS = 128
COLS = 3906
W = COLS + 1
L = ROWS * COLS
NCORES = 8
N = 4000002
D = N - 2
SIG = 0.3989422804014327
SQRT_PI = math.sqrt(math.pi)

BOUNDS = [0, 512, 1024, 2048, 3394, 3906]
NCH = len(BOUNDS) - 1
PBLK = [(0, 2048), (2048, 3906)]
# accT columns: 0 uA 1 vwA 2 qA 3 u2 4 vw2 5 q2 6 p0 7 vw3 8 u34 9 vw4
# 10 q34 11 p1
NACC = 12
U_COLS, VW_COLS, Q_COLS, P_COLS = [0, 3, 8], [1, 4, 7, 9], [2, 5, 10], [6, 11]

_cached = {}


def _chunk_of(col):
    for j in range(NCH):
        if col < BOUNDS[j + 1]:
            return j
    return NCH - 1


def _build_program():
    import concourse.bass as bass
    import concourse.mybir as mybir

    f32 = mybir.dt.float32
    DERF = mybir.ActivationFunctionType.Derivative_Erf
    nc = bass.Bass("TRN2", target_bir_lowering=False, debug=False,
                   num_devices=NCORES)
    d_ins, y_ins = [], []
    for j in range(NCH):
        a, e = BOUNDS[j], BOUNDS[j + 1]
        cw = (W if j == NCH - 1 else e + 1) - a
        d_ins.append(nc.declare_dram_parameter(f"d{j}", [ROWS, cw], f32,
                                               isOutput=False))
        y_ins.append(nc.declare_dram_parameter(f"y{j}", [ROWS, cw], f32,
                                               isOutput=False))
    wid_in = nc.declare_dram_parameter("wid", [ROWS, 2 * ROWS], f32,
                                       isOutput=False)
    acc_out = nc.declare_dram_parameter("acc", [ROWS, NACC], f32,
                                        isOutput=True)

    from contextlib import ExitStack
    with ExitStack() as st:
        dsem = [st.enter_context(nc.semaphore(f"dsem{j}"))
                for j in range(NCH)]
        v_sem = st.enter_context(nc.semaphore("v_sem"))
        w_sem = st.enter_context(nc.semaphore("w_sem"))
        pe_sem = st.enter_context(nc.semaphore("pe_sem"))
        dr_sem = st.enter_context(nc.semaphore("dr_sem"))
        out_sem = st.enter_context(nc.semaphore("out_sem"))
        dt = st.enter_context(nc.sbuf_tensor("dt", [ROWS, W], f32))
        yt = st.enter_context(nc.sbuf_tensor("yt", [ROWS, W], f32))
        ut = st.enter_context(nc.sbuf_tensor("ut", [ROWS, W], f32))
        vwt = st.enter_context(nc.sbuf_tensor("vwt", [ROWS, 2 * W], f32))
        qt = st.enter_context(nc.sbuf_tensor("qt", [ROWS, W], f32))
        wid = st.enter_context(nc.sbuf_tensor("wid_sb", [ROWS, 2 * ROWS], f32))
        sink = st.enter_context(nc.sbuf_tensor("sink", [ROWS, 2 * W], f32))
        accT = st.enter_context(nc.sbuf_tensor("accT", [ROWS, NACC], f32))
        bias0 = st.enter_context(nc.sbuf_tensor("bias0", [ROWS, 1], f32))
        pp = [st.enter_context(nc.psum_tensor(f"pp{i}", [ROWS, b - a], f32))
              for i, (a, b) in enumerate(PBLK)]
        block = st.enter_context(nc.Block())

        def chunk_cols(j):
            a, e = BOUNDS[j], BOUNDS[j + 1]
            return a, (W if j == NCH - 1 else e + 1)

        @block.sync
        def _(sync):
            # wid first: gates the PE; d2 rides the scalar ring instead so
            # ds2 lands ~2us earlier (it was the DVE chunk-2 stall source)
            sync.dma_start(wid[:, :], wid_in[:, :]).then_inc(w_sem, 16)
            for j in (0, 1, 3, 4):
                a, e = chunk_cols(j)
                sync.dma_start(dt[:, a:e], d_ins[j][:, :]) \
                    .then_inc(dsem[j], 16)

        @block.gpsimd
        def _(gpsimd):
            # y chunks on the software-DGE queue: frees the ACT engine of
            # all DMA work
            for j in range(NCH):
                a, e = chunk_cols(j)
                gpsimd.dma_start(yt[:, a:e], y_ins[j][:, :]) \
                      .then_inc(dsem[j], 16)

        @block.scalar
        def _(scalar):
            # d2 on the otherwise-idle scalar HWDGE ring; the issue hides in
            # ACT's pre-data window
            a2, e2 = chunk_cols(2)
            scalar.dma_start(dt[:, a2:e2], d_ins[2][:, :]) \
                  .then_inc(dsem[2], 16)
            # warmup activation: hoists the ~1.3us DERF table load off the
            # critical path (garbage in, output discarded)
            scalar.activation(sink[:, 0:1], bias0[:, 0:1], DERF,
                              bias=bias0[:, 0:1], scale=SQRT_PI)

            def gauss(in_ap, out_ap, col):
                scalar.activation(
                    out_ap, in_ap, DERF, bias=bias0[:, 0:1], scale=SQRT_PI,
                    accum_out=accT[:, col:col + 1])

            def u_act(a, e, wv, col, last=False):
                ue = W if last else e
                scalar.wait_ge(v_sem, wv)
                gauss(ut[:, a:ue], sink[:, a:ue], col)

            def vw_act(a, e, wv, col):
                cw = e - a
                scalar.wait_ge(v_sem, wv)
                vw_in = bass.AP(vwt, a, [[2 * W, ROWS], [W, 2], [1, cw]])
                vw_out = bass.AP(sink, 0, [[2 * W, ROWS], [W, 2], [1, cw]])
                gauss(vw_in, vw_out, col)

            def q_act(a, e, wv, col):
                scalar.wait_ge(v_sem, wv)
                gauss(qt[:, a:e], sink[:, a:e], col)

            # spans sized so every act's v_sem wait is satisfied before ACT
            # reaches it; acts after DVE's finish form a pure backlog, so
            # fewer instructions there directly shortens the end
            u_act(0, 1024, 5, 0)
            vw_act(0, 1024, 7, 1)
            q_act(0, 1024, 8, 2)
            u_act(1024, 2048, 9, 3)
            vw_act(1024, 2048, 11, 4)
            q_act(1024, 2048, 12, 5)
            scalar.wait_ge(pe_sem, 1)
            gauss(pp[0][:, :], sink[:, 0:PBLK[0][1] - PBLK[0][0]], 6)
            vw_act(2048, 3394, 15, 7)
            u_act(2048, 3906, 17, 8, last=True)
            vw_act(3394, 3906, 19, 9)
            q_act(2048, 3906, 20, 10)
            scalar.wait_ge(pe_sem, 2)
            gauss(pp[1][:, :], sink[:, 0:PBLK[1][1] - PBLK[1][0]], 11)
            # drain the ACT pipe so the accums land, then ship the output
            # directly from this engine; the ~1.9us transfer completes under
            # the fixed NEFF epilogue, so nothing waits on it
            scalar.drain()
            scalar.dma_start(acc_out[:, :], accT[:, :]).then_inc(out_sem, 16)
            # scalar reaching this point proves v_sem=20 (so every dsem was
            # consumed) and pe_sem=2 (so w_sem was consumed): clearing all
            # semaphores here is race-free
            for s in dsem:
                scalar.sem_clear(s)
            for s in (v_sem, w_sem, pe_sem, dr_sem, out_sem):
                scalar.sem_clear(s)

        @block.vector
        def _(vector):
            vector.memset(bias0[:, :], 0.0)
            for j in range(NCH):
                a, e = BOUNDS[j], BOUNDS[j + 1]
                ue = W if j == NCH - 1 else e
                vector.wait_ge(dsem[j], 32)
                vector.tensor_sub(ut[:, a:ue], dt[:, a:ue], yt[:, a:ue]) \
                      .then_inc(v_sem, 1)
                vector.tensor_sub(vwt[:, a:e], dt[:, a + 1:e + 1],
                                  yt[:, a:e]).then_inc(v_sem, 1)
                vector.tensor_sub(vwt[:, W + a:W + e], dt[:, a:e],
                                  yt[:, a + 1:e + 1]).then_inc(v_sem, 1)
                vector.tensor_sub(qt[:, a:e], yt[:, a + 1:e + 1],
                                  yt[:, a:e]).then_inc(v_sem, 1)

        @block.tensor
        def _(tensor):
            tensor.wait_ge(w_sem, 16)
            ineg = wid[:, 0:ROWS]
            ipos = wid[:, ROWS:2 * ROWS]
            waited = -1
            for i, (pa, pb) in enumerate(PBLK):
                last = None
                for s in range(pa, pb, 512):
                    sb = min(s + 512, pb)
                    need = _chunk_of(sb - 1)
                    while waited < need:
                        waited += 1
                        tensor.wait_ge(dsem[waited], 32)
                    tensor.matmul(pp[i][:, s - pa:sb - pa],
                                  ineg, dt[:, s:sb],
                                  start=True, stop=False)
                    last = tensor.matmul(
                        pp[i][:, s - pa:sb - pa],
                        ipos, dt[:, s + 1:sb + 1],
                        start=False, stop=True)
                last.then_inc(pe_sem, 1)

    return nc


def _overlap_tiles(x):
    sv = x.strides[0]
    out = []
    for c in range(NCORES):
        base = x[c * L:]
        m = np.lib.stride_tricks.as_strided(
            base, shape=(ROWS, W), strides=(COLS * sv, sv))
        chunks = []
        for j in range(NCH):
            a, e = BOUNDS[j], BOUNDS[j + 1]
            ecap = W if j == NCH - 1 else e + 1
            chunks.append(np.ascontiguousarray(m[:, a:ecap]))
        out.append(chunks)
    return out


def _g64(t):
    t = np.asarray(t, dtype=np.float64)
    return np.exp(-np.pi * t * t)


def make_in_maps(d, y):
    dts = _overlap_tiles(d)
    yts = _overlap_tiles(y)
    wid = np.concatenate([-np.eye(ROWS), np.eye(ROWS)],
                         axis=1).astype(np.float32)
    in_maps = []
    for c in range(NCORES):
        m = {"wid": wid}
        for j in range(NCH):
            m[f"d{j}"] = dts[c][j]
            m[f"y{j}"] = yts[c][j]
        in_maps.append(m)
    return in_maps


def kernel(d, y):
    from concourse.bass_utils import run_bass_kernel_spmd

    d = np.ascontiguousarray(np.asarray(d, dtype=np.float32))
    y = np.ascontiguousarray(np.asarray(y, dtype=np.float32))

    if "nc" not in _cached:
        _cached["nc"] = _build_program()
    nc = _cached["nc"]

    in_maps = make_in_maps(d, y)
    if "warm" not in _cached:
        run_bass_kernel_spmd(nc, in_maps, list(range(NCORES)))
        _cached["warm"] = True
    res = run_bass_kernel_spmd(nc, in_maps, list(range(NCORES))).results

    acc = np.stack([r["acc"] for r in res]).astype(np.float64)  # [8,128,14]
    scale = SQRT_PI / 2.0
    U_dev = acc[:, :, U_COLS].sum() * scale
    VW_dev = acc[:, :, VW_COLS].sum() * scale
    Q_dev = acc[:, :, Q_COLS].sum() * scale
    P_dev = acc[:, :, P_COLS].sum() * scale

    d64 = d.astype(np.float64)
    y64 = y.astype(np.float64)
    cov = NCORES * L

    # u stream covers [0, W) per core: correct row/core duplicates + tail
    dup_idx = []
    for c in range(NCORES):
        dup_idx.extend(c * L + r * COLS for r in range(1, ROWS))
    dup_idx.extend(c * L for c in range(1, NCORES))
    dup_idx = np.asarray(dup_idx)
    u_dup = _g64(d64[dup_idx] - y64[dup_idx]).sum()
    jt = np.arange(cov + 1, D + 1)
    u_tail = _g64(d64[jt] - y64[jt]).sum()
    U = U_dev - u_dup + u_tail

    kt = np.arange(cov, D)
    VW = VW_dev + _g64(d64[kt + 1] - y64[kt]).sum() \
        + _g64(d64[kt] - y64[kt + 1]).sum()
    P = P_dev + _g64(d64[kt + 1] - d64[kt]).sum()
    Q = Q_dev + _g64(y64[kt + 1] - y64[kt]).sum()

    u0 = _g64(d64[0] - y64[0])
    uD = _g64(d64[D] - y64[D])
    S3 = 2.0 * U - u0 - uD + VW
    S1, S2 = Q, P

    lsp32 = np.float32(0.5 * D * (math.log(2.0 * math.pi)
                                  + 2.0 * math.log(SIG)))
    total = math.exp(-float(lsp32)) * (D + (S1 + S2 - S3) / 2.0)
    return np.array(total, dtype=np.float32)
